# revision 8
# baseline (speedup 1.0000x reference)
"""Trainium2 Bass kernel for nn_CombinedLoss (deep-supervision CE + spectrum loss).

V2 strategy (data parallel over batch B=512 -> 64 rows per core; partition
p = 64*h + b, h = s-half):

Spectrum (critical path): logits[5] DMA'd first -> exp/se/expected residue
mass [128,20] -> in-partition cumsum (tensor_tensor_scan) -> cross-partition
fixup via two tiny PE matmuls -> theo ion masses [128,57] -> arithmetic slot
index (host pre-bins peaks into uniform 1.25-Da slots of <=5 peaks) -> one
indirect-DMA gather of 2 consecutive slots per ion (no on-chip searchsorted)
-> windowed softmax/huber on [128, 57*10] in fp16 where safe.

CE (fills the gather gap): host packs one-hot(targets)*mask (bf16) and
w_t*mask (f32) into an aux tensor; on-chip work is exp (ACT, bf16), a
log-sum-exp V-tree in bf16, and 6 multiply-accumulate ops on raw logits.

One activation-table preload (natural_log_exp_and_others) covers
Exp/Ln/Abs/Square, so no mid-kernel table switches.
"""

import os
import sys

import numpy as np

for _p in ("/opt/trn_rl_repo", "/root/.axon_site/_ro/trn_rl_repo"):
    if os.path.isdir(_p) and _p not in sys.path:
        sys.path.append(_p)

T, B, S, V = 6, 512, 40, 28
N_PEAKS = 512
NCORES = 8
BS = B // NCORES          # 64 batch rows per core
PROTON = 1.007276
WATER = 18.010565
CO = 27.994915
MASS_TOL = 0.5
TEMP = 0.1
HUB_D = 0.2
CE_W = 1.0
SPEC_W = 0.1

NION = 57                 # ion slots per partition (19 per family)
DELTA = 1.25              # slot width in Da
MAXP = 5                  # max peaks per slot (asserted host-side)
NSLOT = 1528              # slots per row (covers 100..2010 + pad)
KMAX = float(NSLOT - 2)
RECW = 2 * MAXP           # f32 elems per slot record (5 mass + 5 inten)
WG = 2 * MAXP             # gathered candidate peaks per ion (2 slots)
PADM = 30000.0            # pad mass (stays finite in fp16)
H1 = 28                   # ions in first gather half

_cached = {}


def _build_program():
    import concourse.bass as bass
    import concourse.bacc as bacc
    import concourse.mybir as mybir
    import concourse.tile as tile

    dt = mybir.dt
    Alu = mybir.AluOpType
    Act = mybir.ActivationFunctionType
    AX = mybir.AxisListType

    nc = bacc.Bacc("TRN2", target_bir_lowering=False, debug=False,
                   num_devices=NCORES)

    f32, bf16, fp16 = dt.float32, dt.bfloat16, dt.float16

    lg5_d = nc.dram_tensor("lg5", [128, 20 * V], f32, kind="ExternalInput")
    lgr_d = nc.dram_tensor("lgr", [5, 128, 20 * V], f32, kind="ExternalInput")
    aux_d = nc.dram_tensor("aux", [128, 1728], dt.uint8, kind="ExternalInput")
    slots_d = nc.dram_tensor("slots", [BS * NSLOT, RECW], f32,
                             kind="ExternalInput")
    out_d = nc.dram_tensor("partials", [128, 4], f32, kind="ExternalOutput")

    with tile.TileContext(nc) as tc:
        with tc.tile_pool(name="main", bufs=1) as pool, \
             tc.tile_pool(name="ps", bufs=1, space="PSUM") as psp:

            # ---- act table preload: set 6 = natural_log_exp_and_others ----
            ld = mybir.InstLoadActFuncSet(
                name=nc.get_next_instruction_name(), ins=[], outs=[],
                act_func_set_id=6)
            nc.scalar.add_instruction(ld)

            # ---------------- input DMAs (SP seq; t5 first) ----------------
            lg5 = pool.tile([128, 20, V], f32, tag="lg5")
            nc.sync.dma_start(out=lg5[:].rearrange("p a b -> p (a b)"),
                              in_=lg5_d.ap())
            aux = pool.tile([128, 1728], dt.uint8, tag="aux")
            nc.sync.dma_start(out=aux[:], in_=aux_d.ap())
            lgr = pool.tile([128, 5, 20, V], f32, tag="lgr")
            nc.sync.dma_start(out=lgr[:].rearrange("p t a b -> p t (a b)"),
                              in_=lgr_d.ap().rearrange("t p n -> p t n"))

            ohm = aux[:, 0:1120].bitcast(bf16)            # [128, 560]
            wM = aux[:, 1120:1600].bitcast(f32)           # [128, 120]
            aar = aux[:, 1600:1712].bitcast(f32)          # [128, 28]

            # ---------------- early constants (Pool/DVE) ----------------
            iota_p = pool.tile([128, 1], dt.int32, tag="iota_p")
            nc.gpsimd.iota(iota_p[:], pattern=[[0, 1]], channel_multiplier=1)
            pp_f = pool.tile([128, 1], f32, tag="pp_f")
            nc.vector.tensor_copy(out=pp_f[:], in_=iota_p[:])
            # b = p mod 64 ; base = b * NSLOT (slot-unit gather base)
            ge64 = pool.tile([128, 1], f32, tag="ge64")
            nc.vector.tensor_scalar(out=ge64[:], in0=pp_f[:], scalar1=63.5,
                                    scalar2=64.0, op0=Alu.is_gt, op1=Alu.mult)
            b_f = pool.tile([128, 1], f32, tag="b_f")
            nc.vector.tensor_tensor(out=b_f[:], in0=pp_f[:], in1=ge64[:],
                                    op=Alu.subtract)
            base_f = pool.tile([128, 1], f32, tag="base_f")
            nc.vector.tensor_scalar(out=base_f[:], in0=b_f[:],
                                    scalar1=float(NSLOT), scalar2=None,
                                    op0=Alu.mult)
            # selector matrices for cross-partition cumsum fixup
            iota_n = pool.tile([128, 128], dt.int32, tag="iota_n")
            nc.gpsimd.iota(iota_n[:], pattern=[[1, 128]], channel_multiplier=0)
            iota_nf = pool.tile([128, 128], f32, tag="iota_nf")
            nc.vector.tensor_copy(out=iota_nf[:], in_=iota_n[:])
            p64 = pool.tile([128, 1], f32, tag="p64")
            nc.vector.tensor_scalar(out=p64[:], in0=pp_f[:], scalar1=64.0,
                                    scalar2=None, op0=Alu.add)
            sel1 = pool.tile([128, 128], f32, tag="sel1")
            nc.vector.tensor_scalar(out=sel1[:], in0=iota_nf[:],
                                    scalar1=p64[:], scalar2=None,
                                    op0=Alu.is_equal)
            # sel2[p', n] = 1 iff n == p' - 64  (h1 row p' feeds h0 column n)
            pm64 = pool.tile([128, 1], f32, tag="pm64")
            nc.vector.tensor_scalar(out=pm64[:], in0=pp_f[:], scalar1=-64.0,
                                    scalar2=None, op0=Alu.add)
            sel2 = pool.tile([128, 128], f32, tag="sel2")
            nc.vector.tensor_scalar(out=sel2[:], in0=iota_nf[:],
                                    scalar1=pm64[:], scalar2=None,
                                    op0=Alu.is_equal)

            theo = pool.tile([128, NION], f32, tag="theo")
            nc.vector.memset(theo[:], -PADM)

            # ---------------- spectrum front (t=5) ----------------
            e3_5 = pool.tile([128, 20, V], f32, tag="e3_5")
            nc.scalar.activation(out=e3_5[:], in_=lg5[:], func=Act.Exp)
            se5 = pool.tile([128, 20], f32, tag="se5")
            nc.vector.tensor_reduce(out=se5[:], in_=e3_5[:], axis=AX.X,
                                    op=Alu.add)
            pe5 = pool.tile([128, 20], f32, tag="pe5")
            nc.vector.reciprocal(out=pe5[:], in_=se5[:])
            prod5 = pool.tile([128, 20, V], f32, tag="prod5")
            nc.vector.tensor_tensor(
                out=prod5[:], in0=e3_5[:],
                in1=aar[:, None, :].broadcast_to([128, 20, V]), op=Alu.mult)
            nume = pool.tile([128, 20], f32, tag="nume")
            nc.vector.tensor_reduce(out=nume[:], in_=prod5[:], axis=AX.X,
                                    op=Alu.add)
            expected = pool.tile([128, 20], f32, tag="expected")
            nc.vector.tensor_tensor(out=expected[:], in0=nume[:], in1=pe5[:],
                                    op=Alu.mult)
            # residues are s=1..38: zero s=0 (h0 col 0) and s=39 (h1 col 19)
            nc.vector.memset(expected[0:64, 0:1], 0.0)
            nc.vector.memset(expected[64:128, 19:20], 0.0)

            # in-partition cumsum over the 20 s-slots
            zer20 = pool.tile([128, 20], f32, tag="zer20")
            nc.vector.memset(zer20[:], 0.0)
            cum = pool.tile([128, 20], f32, tag="cum")
            nc.vector.tensor_tensor_scan(out=cum[:], data0=expected[:],
                                         data1=zer20[:], initial=0.0,
                                         op0=Alu.add, op1=Alu.add)

            # cross-partition fixups: fix[:,0] = prev-half total (h1 rows),
            # fix[:,1] = per-b grand total
            fix_ps = psp.tile([128, 2], f32, tag="fix_ps")
            nc.tensor.matmul(out=fix_ps[:, 0:1], lhsT=sel1[:],
                             rhs=cum[:, 19:20], start=True, stop=True)
            nc.tensor.matmul(out=fix_ps[:, 1:2], lhsT=sel2[:],
                             rhs=cum[:, 19:20], start=True, stop=True)
            fix = pool.tile([128, 2], f32, tag="fix")
            nc.scalar.copy(out=fix[:], in_=fix_ps[:])
            cfull = pool.tile([128, 20], f32, tag="cfull")
            nc.vector.tensor_scalar(out=cfull[:], in0=cum[:],
                                    scalar1=fix[:, 0:1], scalar2=None,
                                    op0=Alu.add)
            # grand total c37 = own local sum + partner-half local sum
            tsum = pool.tile([128, 1], f32, tag="tsum")
            nc.vector.tensor_tensor(out=tsum[:], in0=fix[:, 0:1],
                                    in1=fix[:, 1:2], op=Alu.add)
            totWP = pool.tile([128, 1], f32, tag="totWP")
            nc.vector.tensor_scalar(out=totWP[:], in0=tsum[:],
                                    scalar1=cum[:, 19:20],
                                    scalar2=WATER + PROTON,
                                    op0=Alu.add, op1=Alu.add)

            # theo[0:19]=b-ions, [19:38]=y-ions, [38:57]=a-ions
            nc.vector.tensor_scalar(out=theo[0:64, 0:19],
                                    in0=cfull[0:64, 1:20], scalar1=PROTON,
                                    scalar2=None, op0=Alu.add)
            nc.vector.tensor_scalar(out=theo[64:128, 0:18],
                                    in0=cfull[64:128, 0:18], scalar1=PROTON,
                                    scalar2=None, op0=Alu.add)
            # y_q = totWP - c_{q-1}: h0 uses cfull[:, 0:19] (col 0 == 0);
            # h1 slot 0 (q=19) uses c_18 = fix1, slots 1..17 use cfull[0:17]
            nc.vector.tensor_scalar(out=theo[0:64, 19:38],
                                    in0=cfull[0:64, 0:19], scalar1=-1.0,
                                    scalar2=totWP[0:64], op0=Alu.mult,
                                    op1=Alu.add)
            nc.vector.tensor_scalar(out=theo[64:128, 19:20],
                                    in0=fix[64:128, 0:1], scalar1=-1.0,
                                    scalar2=totWP[64:128], op0=Alu.mult,
                                    op1=Alu.add)
            nc.vector.tensor_scalar(out=theo[64:128, 20:37],
                                    in0=cfull[64:128, 0:17], scalar1=-1.0,
                                    scalar2=totWP[64:128], op0=Alu.mult,
                                    op1=Alu.add)
            nc.vector.tensor_scalar(out=theo[0:64, 38:57],
                                    in0=cfull[0:64, 1:20],
                                    scalar1=PROTON - CO, scalar2=None,
                                    op0=Alu.add)
            nc.vector.tensor_scalar(out=theo[64:128, 38:56],
                                    in0=cfull[64:128, 0:18],
                                    scalar1=PROTON - CO, scalar2=None,
                                    op0=Alu.add)

            # ---------------- slot index + gather ----------------
            kf = pool.tile([128, NION], f32, tag="kf")
            nc.vector.tensor_scalar(out=kf[:], in0=theo[:],
                                    scalar1=1.0 / DELTA,
                                    scalar2=-(100.0 + MASS_TOL) / DELTA,
                                    op0=Alu.mult, op1=Alu.add)
            kc = pool.tile([128, NION], f32, tag="kc")
            nc.vector.tensor_scalar(out=kc[:], in0=kf[:], scalar1=0.0,
                                    scalar2=KMAX, op0=Alu.max, op1=Alu.min)
            off_f = pool.tile([128, NION], f32, tag="off_f")
            nc.vector.tensor_scalar(out=off_f[:], in0=kc[:],
                                    scalar1=base_f[:], scalar2=None,
                                    op0=Alu.add)
            off_u = pool.tile([128, NION], dt.uint32, tag="off_u")
            nc.vector.tensor_copy(out=off_u[:], in_=off_f[:])

            cmpt = pool.tile([128, NION, 2 * RECW], f32, tag="cmpt")
            gathers = []
            for h0, sl in ((0, slice(0, H1)), (1, slice(H1, NION))):
                g = nc.gpsimd.indirect_dma_start(
                    out=cmpt[:, sl].rearrange("p a b -> p (a b)"),
                    out_offset=None,
                    in_=slots_d.ap(),
                    in_offset=bass.IndirectOffsetOnAxis(ap=off_u[:, sl],
                                                        axis=0))
                gathers.append(g)

            # ---------------- CE (fills the gather gap) ----------------
            e3r = pool.tile([128, 5, 20, V], bf16, tag="e3r")
            nc.scalar.activation(
                out=e3r[:].rearrange("p t a b -> p (t a b)"),
                in_=lgr[:].rearrange("p t a b -> p (t a b)"), func=Act.Exp)
            # V-tree for log-sum-exp (bf16): 28 -> 14 -> 7 -> reduce
            t14 = pool.tile([128, 5, 20, 14], bf16, tag="t14")
            nc.vector.tensor_tensor(out=t14[:], in0=e3r[:, :, :, 0:14],
                                    in1=e3r[:, :, :, 14:28], op=Alu.add)
            t7 = pool.tile([128, 5, 20, 7], bf16, tag="t7")
            nc.vector.tensor_tensor(out=t7[:], in0=t14[:, :, :, 0:7],
                                    in1=t14[:, :, :, 7:14], op=Alu.add)
            se6 = pool.tile([128, 6, 20], f32, tag="se6")
            nc.vector.tensor_reduce(out=se6[:, 0:5], in_=t7[:], axis=AX.X,
                                    op=Alu.add)
            nc.vector.tensor_copy(out=se6[:, 5], in_=se5[:])
            lse = pool.tile([128, 6, 20], f32, tag="lse")
            nc.scalar.activation(out=lse[:].rearrange("p a b -> p (a b)"),
                                 in_=se6[:].rearrange("p a b -> p (a b)"),
                                 func=Act.Ln)
            partials = pool.tile([128, 4], f32, tag="partials")
            junkA = pool.tile([128, 6, 20], f32, tag="junkA")
            ce1 = pool.tile([128, 1], f32, tag="ce1")
            nc.vector.scalar_tensor_tensor(
                out=junkA[:].rearrange("p a b -> p (a b)"),
                in0=lse[:].rearrange("p a b -> p (a b)"), scalar=1.0,
                in1=wM[:], op0=Alu.mult, op1=Alu.mult, accum_out=ce1[:])
            # x-part: sum_t w_t * sum_{s,v} logits * onehot*mask
            W6 = [(i + 1) / 21.0 for i in range(T)]
            ce2c = pool.tile([128, 6], f32, tag="ce2c")
            junkB = pool.tile([128, 20, V], f32, tag="junkB")
            for t in range(T):
                src = lg5[:] if t == 5 else lgr[:, t]
                nc.vector.scalar_tensor_tensor(
                    out=junkB[:].rearrange("p a b -> p (a b)"),
                    in0=src.rearrange("p a b -> p (a b)"), scalar=W6[t],
                    in1=ohm[:], op0=Alu.mult, op1=Alu.mult,
                    accum_out=ce2c[:, t:t + 1])
            ce2 = pool.tile([128, 1], f32, tag="ce2")
            nc.vector.tensor_reduce(out=ce2[:], in_=ce2c[:], axis=AX.X,
                                    op=Alu.add)
            nc.vector.tensor_tensor(out=partials[:, 0:1], in0=ce1[:],
                                    in1=ce2[:], op=Alu.subtract)
            nc.vector.memset(partials[:, 1:2], 0.0)

            # ---------------- S3: windowed softmax/huber ----------------
            den = pool.tile([128, NION], fp16, tag="den")
            hubn = pool.tile([128, NION], fp16, tag="hubn")
            iwn = pool.tile([128, NION], fp16, tag="iwn")
            for h0, sl in ((0, slice(0, H1)), (1, slice(H1, NION))):
                n = sl.stop - sl.start
                cm = cmpt[:, sl].rearrange("p a (r m) -> p a r m", r=2)
                og = cm[:, :, :, 0:MAXP]
                ig = cm[:, :, :, MAXP:RECW]
                theoB = theo[:, sl][:, :, None, None].broadcast_to(
                    [128, n, 2, MAXP])
                d0 = pool.tile([128, n, 2, MAXP], fp16, tag=f"d0_{h0}")
                nc.vector.tensor_tensor(out=d0[:], in0=og, in1=theoB,
                                        op=Alu.subtract)
                dd = pool.tile([128, n, 2, MAXP], fp16, tag=f"dd_{h0}")
                nc.scalar.activation(out=dd[:].rearrange("p a r m -> p (a r m)"),
                                     in_=d0[:].rearrange("p a r m -> p (a r m)"),
                                     func=Act.Abs)
                ee = pool.tile([128, n, 2, MAXP], fp16, tag=f"ee_{h0}")
                nc.scalar.activation(out=ee[:].rearrange("p a r m -> p (a r m)"),
                                     in_=dd[:].rearrange("p a r m -> p (a r m)"),
                                     func=Act.Exp, scale=-1.0 / TEMP)
                mw = pool.tile([128, n, 2, MAXP], fp16, tag=f"mw_{h0}")
                nc.vector.tensor_scalar(out=mw[:], in0=dd[:],
                                        scalar1=MASS_TOL, scalar2=None,
                                        op0=Alu.is_lt)
                ew = pool.tile([128, n, 2, MAXP], fp16, tag=f"ew_{h0}")
                nc.vector.tensor_tensor(out=ew[:], in0=mw[:], in1=ee[:],
                                        op=Alu.mult)
                with nc.allow_low_precision(reason="short fp16 window sums"):
                    nc.vector.tensor_reduce(
                        out=den[:, sl],
                        in_=ew[:].rearrange("p a r m -> p a (r m)"),
                        axis=AX.X, op=Alu.add)
                c1 = pool.tile([128, n, 2, MAXP], fp16, tag=f"c1_{h0}")
                nc.vector.tensor_scalar(out=c1[:], in0=dd[:], scalar1=HUB_D,
                                        scalar2=float(np.sqrt(0.5)),
                                        op0=Alu.min, op1=Alu.mult)
                hm = pool.tile([128, n, 2, MAXP], fp16, tag=f"hm_{h0}")
                nc.scalar.activation(out=hm[:].rearrange("p a r m -> p (a r m)"),
                                     in_=c1[:].rearrange("p a r m -> p (a r m)"),
                                     func=Act.Square)
                rr = pool.tile([128, n, 2, MAXP], fp16, tag=f"rr_{h0}")
                nc.vector.tensor_scalar(out=rr[:], in0=dd[:], scalar1=HUB_D,
                                        scalar2=-HUB_D * HUB_D, op0=Alu.mult,
                                        op1=Alu.add)
                rrm = pool.tile([128, n, 2, MAXP], fp16, tag=f"rrm_{h0}")
                nc.vector.tensor_scalar(out=rrm[:], in0=rr[:], scalar1=0.0,
                                        scalar2=HUB_D * (MASS_TOL - HUB_D),
                                        op0=Alu.max, op1=Alu.min)
                hub = pool.tile([128, n, 2, MAXP], fp16, tag=f"hub_{h0}")
                nc.vector.tensor_tensor(out=hub[:], in0=rrm[:], in1=hm[:],
                                        op=Alu.add)
                he = pool.tile([128, n, 2, MAXP], fp16, tag=f"he_{h0}")
                nc.vector.tensor_tensor(out=he[:], in0=ew[:], in1=hub[:],
                                        op=Alu.mult)
                ie = pool.tile([128, n, 2, MAXP], fp16, tag=f"ie_{h0}")
                nc.vector.tensor_tensor(out=ie[:], in0=ew[:], in1=ig,
                                        op=Alu.mult)
                with nc.allow_low_precision(reason="short fp16 window sums"):
                    nc.vector.tensor_reduce(
                        out=hubn[:, sl],
                        in_=he[:].rearrange("p a r m -> p a (r m)"),
                        axis=AX.X, op=Alu.add)
                    nc.vector.tensor_reduce(
                        out=iwn[:, sl],
                        in_=ie[:].rearrange("p a r m -> p a (r m)"),
                        axis=AX.X, op=Alu.add)

            # ---------------- S4: per-ion contributions ----------------
            nm = pool.tile([128, NION], f32, tag="nm")
            nc.vector.tensor_scalar(out=nm[:], in0=den[:], scalar1=0.0,
                                    scalar2=None, op0=Alu.is_gt)
            dsafe = pool.tile([128, NION], f32, tag="dsafe")
            nc.vector.tensor_scalar(out=dsafe[:], in0=den[:], scalar1=1e-12,
                                    scalar2=None, op0=Alu.max)
            rec = pool.tile([128, NION], f32, tag="rec")
            nc.vector.reciprocal(out=rec[:], in_=dsafe[:])
            t1 = pool.tile([128, NION], f32, tag="t1")
            nc.vector.tensor_tensor(out=t1[:], in0=hubn[:], in1=iwn[:],
                                    op=Alu.mult)
            t2 = pool.tile([128, NION], f32, tag="t2")
            nc.vector.tensor_tensor(out=t2[:], in0=t1[:], in1=rec[:],
                                    op=Alu.mult)
            t3 = pool.tile([128, NION], f32, tag="t3")
            nc.vector.tensor_tensor(out=t3[:], in0=t2[:], in1=rec[:],
                                    op=Alu.mult)
            junkC = pool.tile([128, NION], f32, tag="junkC")
            nc.vector.scalar_tensor_tensor(
                out=junkC[:], in0=t3[:], scalar=1.0, in1=nm[:],
                op0=Alu.mult, op1=Alu.mult, accum_out=partials[:, 2:3])
            nc.vector.tensor_reduce(out=partials[:, 3:4], in_=nm[:],
                                    axis=AX.X, op=Alu.add)

            # ---------------- output ----------------
            nc.sync.dma_start(out=out_d.ap(), in_=partials[:])

    nc.compile()
    return nc


def _get_nc():
    if "nc" not in _cached:
        _cached["nc"] = _build_program()
    return _cached["nc"]


def _host_prep(all_logits, targets, target_mask, observed_masses,
               observed_intensities, peak_mask, aa_masses):
    """Shard + massage inputs into per-core input maps."""
    all_logits = np.ascontiguousarray(all_logits, dtype=np.float32)
    targets = np.asarray(targets, dtype=np.int32)
    tmask = np.asarray(target_mask).astype(np.float32)
    obs = np.asarray(observed_masses, dtype=np.float32)
    inten = np.asarray(observed_intensities, dtype=np.float32)
    pmask = np.asarray(peak_mask).astype(bool)
    aa = np.asarray(aa_masses, dtype=np.float32)

    # ---- uniform slot table: per row, slot k covers [100+k*D, 100+(k+1)*D)
    rows, cols = np.nonzero(pmask)
    m = obs[rows, cols]
    k = np.floor((m - 100.0) / DELTA).astype(np.int64)
    np.clip(k, 0, NSLOT - 1, out=k)
    flat = rows * NSLOT + k
    # masses sorted per row => flat is sorted; rank within each slot:
    first = np.searchsorted(flat, flat, side="left")
    pos = np.arange(flat.size) - first
    if pos.max() >= MAXP:
        raise RuntimeError(f"slot overflow: {pos.max() + 1} peaks in one "
                           f"{DELTA}-Da slot (MAXP={MAXP})")
    slots = np.full((B * NSLOT, RECW), 0.0, dtype=np.float32)
    slots[:, 0:MAXP] = PADM
    slots[flat, pos] = m
    slots[flat, MAXP + pos] = inten[rows, cols]

    # ---- aux: onehot(targets)*mask (bf16) + w_t*mask (f32) + aa (f32)
    oh = (np.arange(V)[None, None, :] == targets[:, :, None])
    nz = (targets != 0)
    mf = tmask
    ohm = (oh * (mf * nz)[:, :, None]).astype(np.float32)  # [B, S, V]
    w6 = (np.arange(1, T + 1, dtype=np.float32) / 21.0)
    wMf = w6[None, :, None] * (mf * nz)[:, None, :]        # [B, T, S]

    def to_bf16_bytes(x):
        u = x.astype(np.float32).view(np.uint32)
        r = ((u >> 16) + ((u >> 15) & 1)).astype(np.uint16)  # round-nearest-ish
        return r

    denom = max(float(mf.sum()), 1.0)

    in_maps = []
    for c in range(NCORES):
        sl = slice(c * BS, (c + 1) * BS)
        lg = all_logits[:, sl]                     # [T, 64, 40, 28]
        # partition p = 64*h + b  (h = s-half)
        lgp = lg.reshape(T, BS, 2, 20, V).transpose(0, 2, 1, 3, 4)
        lgp = np.ascontiguousarray(lgp.reshape(T, 128, 20 * V))
        ohm_c = ohm[sl].reshape(BS, 2, 20, V).transpose(1, 0, 2, 3)
        ohm16 = to_bf16_bytes(ohm_c.reshape(128, 560))
        wM_c = wMf[sl].reshape(BS, T, 2, 20).transpose(2, 0, 1, 3)
        wM_c = np.ascontiguousarray(wM_c.reshape(128, 120), dtype=np.float32)
        aux = np.zeros((128, 1728), dtype=np.uint8)
        aux[:, 0:1120] = ohm16.view(np.uint8).reshape(128, 1120)
        aux[:, 1120:1600] = wM_c.view(np.uint8)
        aux[:, 1600:1712] = np.broadcast_to(
            aa[None, :].view(np.uint8).reshape(1, 112), (128, 112))
        in_maps.append({
            "lg5": np.ascontiguousarray(lgp[5]),
            "lgr": np.ascontiguousarray(lgp[0:5]),
            "aux": aux,
            "slots": np.ascontiguousarray(
                slots.reshape(B, NSLOT, RECW)[sl].reshape(BS * NSLOT, RECW)),
        })
    return in_maps, denom


def _combine(results, denom):
    ce_num = 0.0
    sp_num = 0.0
    sp_cnt = 0.0
    for r in results:
        p = r["partials"].astype(np.float64)
        ce_num += p[:, 0].sum()
        sp_num += p[:, 2].sum()
        sp_cnt += p[:, 3].sum()
    ce = ce_num / denom
    spec = sp_num / max(sp_cnt, 1.0)
    return np.float32(CE_W * ce + SPEC_W * spec)


def kernel(**inputs) -> np.ndarray:
    from concourse.bass_utils import run_bass_kernel_spmd

    nc = _get_nc()
    in_maps, denom = _host_prep(**inputs)
    res = run_bass_kernel_spmd(nc, in_maps, core_ids=list(range(NCORES)))
    return _combine(res.results, denom)


# revision 11
# speedup vs baseline: 1.1566x; 1.1566x over previous
"""Trainium2 Bass kernel for nn_CombinedLoss (deep-supervision CE + spectrum loss).

V2 strategy (data parallel over batch B=512 -> 64 rows per core; partition
p = 64*h + b, h = s-half):

Spectrum (critical path): logits[5] DMA'd first -> exp/se/expected residue
mass [128,20] -> in-partition cumsum (tensor_tensor_scan) -> cross-partition
fixup via two tiny PE matmuls -> theo ion masses [128,57] -> arithmetic slot
index (host pre-bins peaks into uniform 1.25-Da slots of <=5 peaks) -> one
indirect-DMA gather of 2 consecutive slots per ion (no on-chip searchsorted)
-> windowed softmax/huber on [128, 57*10] in fp16 where safe.

CE (fills the gather gap): host packs one-hot(targets)*mask (bf16) and
w_t*mask (f32) into an aux tensor; on-chip work is exp (ACT, bf16), a
log-sum-exp V-tree in bf16, and 6 multiply-accumulate ops on raw logits.

One activation-table preload (natural_log_exp_and_others) covers
Exp/Ln/Abs/Square, so no mid-kernel table switches.
"""

import os
import sys

import numpy as np

for _p in ("/opt/trn_rl_repo", "/root/.axon_site/_ro/trn_rl_repo"):
    if os.path.isdir(_p) and _p not in sys.path:
        sys.path.append(_p)

T, B, S, V = 6, 512, 40, 28
N_PEAKS = 512
NCORES = 8
BS = B // NCORES          # 64 batch rows per core
PROTON = 1.007276
WATER = 18.010565
CO = 27.994915
MASS_TOL = 0.5
TEMP = 0.1
HUB_D = 0.2
CE_W = 1.0
SPEC_W = 0.1

NION = 57                 # ion slots per partition (19 per family)
DELTA = 1.25              # slot width in Da
MAXP = 5                  # max peaks per slot (asserted host-side)
NSLOT = 1528              # slots per row (covers 100..2010 + pad)
KMAX = float(NSLOT - 2)
RECW = 2 * MAXP           # f32 elems per slot record (5 mass + 5 inten)
WG = 2 * MAXP             # gathered candidate peaks per ion (2 slots)
PADM = 30000.0            # pad mass (stays finite in fp16)
H1 = 28                   # ions in first gather half

_cached = {}


def _build_program():
    import concourse.bass as bass
    import concourse.bacc as bacc
    import concourse.mybir as mybir
    import concourse.tile as tile

    dt = mybir.dt
    Alu = mybir.AluOpType
    Act = mybir.ActivationFunctionType
    AX = mybir.AxisListType

    nc = bacc.Bacc("TRN2", target_bir_lowering=False, debug=False,
                   num_devices=NCORES)

    f32, bf16, fp16 = dt.float32, dt.bfloat16, dt.float16

    lg5_d = nc.dram_tensor("lg5", [128, 20 * V], f32, kind="ExternalInput")
    lgr_d = nc.dram_tensor("lgr", [5, 128, 20 * V], f32, kind="ExternalInput")
    aux_d = nc.dram_tensor("aux", [128, 1728], dt.uint8, kind="ExternalInput")
    slots_d = nc.dram_tensor("slots", [BS * NSLOT, 2 * RECW], f32,
                             kind="ExternalInput")
    out_d = nc.dram_tensor("partials", [128, 4], f32, kind="ExternalOutput")

    with tile.TileContext(nc) as tc:
        with tc.tile_pool(name="main", bufs=1) as pool, \
             tc.tile_pool(name="ps", bufs=1, space="PSUM") as psp:

            # ---- act table preload: set 6 = natural_log_exp_and_others ----
            ld = mybir.InstLoadActFuncSet(
                name=nc.get_next_instruction_name(), ins=[], outs=[],
                act_func_set_id=6)
            nc.scalar.add_instruction(ld)

            # ---------------- input DMAs (SP seq; t5 first) ----------------
            lg5 = pool.tile([128, 20, V], f32, tag="lg5")
            nc.sync.dma_start(out=lg5[:].rearrange("p a b -> p (a b)"),
                              in_=lg5_d.ap())
            aux = pool.tile([128, 1728], dt.uint8, tag="aux")
            nc.sync.dma_start(out=aux[:], in_=aux_d.ap())
            lgr = pool.tile([128, 5, 20, V], f32, tag="lgr")
            nc.sync.dma_start(out=lgr[:].rearrange("p t a b -> p t (a b)"),
                              in_=lgr_d.ap().rearrange("t p n -> p t n"))

            ohm = aux[:, 0:1120].bitcast(bf16)            # [128, 560]
            wM = aux[:, 1120:1600].bitcast(f32)           # [128, 120]
            aar = aux[:, 1600:1712].bitcast(f32)          # [128, 28]

            # ---------------- early constants (Pool/DVE) ----------------
            iota_p = pool.tile([128, 1], dt.int32, tag="iota_p")
            nc.gpsimd.iota(iota_p[:], pattern=[[0, 1]], channel_multiplier=1)
            pp_f = pool.tile([128, 1], f32, tag="pp_f")
            nc.vector.tensor_copy(out=pp_f[:], in_=iota_p[:])
            # b = p mod 64 ; base = b * NSLOT (slot-unit gather base)
            ge64 = pool.tile([128, 1], f32, tag="ge64")
            nc.vector.tensor_scalar(out=ge64[:], in0=pp_f[:], scalar1=63.5,
                                    scalar2=64.0, op0=Alu.is_gt, op1=Alu.mult)
            b_f = pool.tile([128, 1], f32, tag="b_f")
            nc.vector.tensor_tensor(out=b_f[:], in0=pp_f[:], in1=ge64[:],
                                    op=Alu.subtract)
            base_f = pool.tile([128, 1], f32, tag="base_f")
            nc.vector.tensor_scalar(out=base_f[:], in0=b_f[:],
                                    scalar1=float(NSLOT), scalar2=None,
                                    op0=Alu.mult)
            # selector matrices for cross-partition cumsum fixup
            iota_n = pool.tile([128, 128], dt.int32, tag="iota_n")
            nc.gpsimd.iota(iota_n[:], pattern=[[1, 128]], channel_multiplier=0)
            iota_nf = pool.tile([128, 128], f32, tag="iota_nf")
            nc.vector.tensor_copy(out=iota_nf[:], in_=iota_n[:])
            p64 = pool.tile([128, 1], f32, tag="p64")
            nc.vector.tensor_scalar(out=p64[:], in0=pp_f[:], scalar1=64.0,
                                    scalar2=None, op0=Alu.add)
            sel1 = pool.tile([128, 128], f32, tag="sel1")
            nc.vector.tensor_scalar(out=sel1[:], in0=iota_nf[:],
                                    scalar1=p64[:], scalar2=None,
                                    op0=Alu.is_equal)
            # sel2[p', n] = 1 iff n == p' - 64  (h1 row p' feeds h0 column n)
            pm64 = pool.tile([128, 1], f32, tag="pm64")
            nc.vector.tensor_scalar(out=pm64[:], in0=pp_f[:], scalar1=-64.0,
                                    scalar2=None, op0=Alu.add)
            sel2 = pool.tile([128, 128], f32, tag="sel2")
            nc.vector.tensor_scalar(out=sel2[:], in0=iota_nf[:],
                                    scalar1=pm64[:], scalar2=None,
                                    op0=Alu.is_equal)

            theo = pool.tile([128, NION], f32, tag="theo")
            nc.vector.memset(theo[:], -PADM)

            # ---------------- spectrum front (t=5) ----------------
            e3_5 = pool.tile([128, 20, V], f32, tag="e3_5")
            nc.scalar.activation(out=e3_5[:], in_=lg5[:], func=Act.Exp)
            se5 = pool.tile([128, 20], f32, tag="se5")
            nc.vector.tensor_reduce(out=se5[:], in_=e3_5[:], axis=AX.X,
                                    op=Alu.add)
            pe5 = pool.tile([128, 20], f32, tag="pe5")
            nc.vector.reciprocal(out=pe5[:], in_=se5[:])
            prod5 = pool.tile([128, 20, V], f32, tag="prod5")
            nc.gpsimd.tensor_tensor(
                out=prod5[:], in0=e3_5[:],
                in1=aar[:, None, :].broadcast_to([128, 20, V]), op=Alu.mult)
            nume = pool.tile([128, 20], f32, tag="nume")
            nc.vector.tensor_reduce(out=nume[:], in_=prod5[:], axis=AX.X,
                                    op=Alu.add)
            expected = pool.tile([128, 20], f32, tag="expected")
            nc.vector.tensor_tensor(out=expected[:], in0=nume[:], in1=pe5[:],
                                    op=Alu.mult)
            # residues are s=1..38: zero s=0 (h0 col 0) and s=39 (h1 col 19)
            nc.vector.memset(expected[0:64, 0:1], 0.0)
            nc.vector.memset(expected[64:128, 19:20], 0.0)

            # in-partition cumsum over the 20 s-slots
            zer20 = pool.tile([128, 20], f32, tag="zer20")
            nc.vector.memset(zer20[:], 0.0)
            cum = pool.tile([128, 20], f32, tag="cum")
            nc.vector.tensor_tensor_scan(out=cum[:], data0=expected[:],
                                         data1=zer20[:], initial=0.0,
                                         op0=Alu.add, op1=Alu.add)

            # cross-partition fixups: fix[:,0] = prev-half total (h1 rows),
            # fix[:,1] = per-b grand total
            fix_ps = psp.tile([128, 2], f32, tag="fix_ps")
            nc.tensor.matmul(out=fix_ps[:, 0:1], lhsT=sel1[:],
                             rhs=cum[:, 19:20], start=True, stop=True)
            nc.tensor.matmul(out=fix_ps[:, 1:2], lhsT=sel2[:],
                             rhs=cum[:, 19:20], start=True, stop=True)
            fix = pool.tile([128, 2], f32, tag="fix")
            nc.scalar.copy(out=fix[:], in_=fix_ps[:])
            cfull = pool.tile([128, 20], f32, tag="cfull")
            nc.vector.tensor_scalar(out=cfull[:], in0=cum[:],
                                    scalar1=fix[:, 0:1], scalar2=None,
                                    op0=Alu.add)
            # grand total c37 = own local sum + partner-half local sum
            tsum = pool.tile([128, 1], f32, tag="tsum")
            nc.vector.tensor_tensor(out=tsum[:], in0=fix[:, 0:1],
                                    in1=fix[:, 1:2], op=Alu.add)
            totWP = pool.tile([128, 1], f32, tag="totWP")
            nc.vector.tensor_scalar(out=totWP[:], in0=tsum[:],
                                    scalar1=cum[:, 19:20],
                                    scalar2=WATER + PROTON,
                                    op0=Alu.add, op1=Alu.add)

            # theo[0:19]=b-ions, [19:38]=y-ions, [38:57]=a-ions
            nc.vector.tensor_scalar(out=theo[0:64, 0:19],
                                    in0=cfull[0:64, 1:20], scalar1=PROTON,
                                    scalar2=None, op0=Alu.add)
            nc.vector.tensor_scalar(out=theo[64:128, 0:18],
                                    in0=cfull[64:128, 0:18], scalar1=PROTON,
                                    scalar2=None, op0=Alu.add)
            # y_q = totWP - c_{q-1}: h0 uses cfull[:, 0:19] (col 0 == 0);
            # h1 slot 0 (q=19) uses c_18 = fix1, slots 1..17 use cfull[0:17]
            nc.vector.tensor_scalar(out=theo[0:64, 19:38],
                                    in0=cfull[0:64, 0:19], scalar1=-1.0,
                                    scalar2=totWP[0:64], op0=Alu.mult,
                                    op1=Alu.add)
            nc.vector.tensor_scalar(out=theo[64:128, 19:20],
                                    in0=fix[64:128, 0:1], scalar1=-1.0,
                                    scalar2=totWP[64:128], op0=Alu.mult,
                                    op1=Alu.add)
            nc.vector.tensor_scalar(out=theo[64:128, 20:37],
                                    in0=cfull[64:128, 0:17], scalar1=-1.0,
                                    scalar2=totWP[64:128], op0=Alu.mult,
                                    op1=Alu.add)
            nc.vector.tensor_scalar(out=theo[0:64, 38:57],
                                    in0=cfull[0:64, 1:20],
                                    scalar1=PROTON - CO, scalar2=None,
                                    op0=Alu.add)
            nc.vector.tensor_scalar(out=theo[64:128, 38:56],
                                    in0=cfull[64:128, 0:18],
                                    scalar1=PROTON - CO, scalar2=None,
                                    op0=Alu.add)

            # ---------------- slot index + gather ----------------
            kf = pool.tile([128, NION], f32, tag="kf")
            nc.vector.tensor_scalar(out=kf[:], in0=theo[:],
                                    scalar1=1.0 / DELTA,
                                    scalar2=-(100.0 + MASS_TOL) / DELTA,
                                    op0=Alu.mult, op1=Alu.add)
            kc = pool.tile([128, NION], f32, tag="kc")
            nc.vector.tensor_scalar(out=kc[:], in0=kf[:], scalar1=0.0,
                                    scalar2=KMAX, op0=Alu.max, op1=Alu.min)
            off_f = pool.tile([128, NION], f32, tag="off_f")
            nc.vector.tensor_scalar(out=off_f[:], in0=kc[:],
                                    scalar1=base_f[:], scalar2=None,
                                    op0=Alu.add)
            off_u = pool.tile([128, NION], dt.uint32, tag="off_u")
            i_offu = nc.vector.tensor_copy(out=off_u[:], in_=off_f[:])

            cmpt = pool.tile([128, NION, 2 * RECW], f32, tag="cmpt")
            gathers = []
            for h0, sl in ((0, slice(0, H1)), (1, slice(H1, NION))):
                g = nc.gpsimd.indirect_dma_start(
                    out=cmpt[:, sl].rearrange("p a b -> p (a b)"),
                    out_offset=None,
                    in_=slots_d.ap(),
                    in_offset=bass.IndirectOffsetOnAxis(ap=off_u[:, sl],
                                                        axis=0))
                gathers.append(g)

            # ---------------- CE (fills the gather gap) ----------------
            from concourse.tile import add_dep_helper
            e3r = pool.tile([128, 5, 20, V], bf16, tag="e3r")
            nc.scalar.activation(
                out=e3r[:].rearrange("p t a b -> p (t a b)"),
                in_=lgr[:].rearrange("p t a b -> p (t a b)"), func=Act.Exp)
            # x-part: sum_t w_t * sum_{s,v} logits * onehot*mask
            W6 = [(i + 1) / 21.0 for i in range(T)]
            ce2c = pool.tile([128, 6], f32, tag="ce2c")
            junkB = pool.tile([128, 20, V], f32, tag="junkB")
            for t in range(T):
                srcl = lg5[:] if t == 5 else lgr[:, t]
                i_acc = nc.vector.scalar_tensor_tensor(
                    out=junkB[:].rearrange("p a b -> p (a b)"),
                    in0=srcl.rearrange("p a b -> p (a b)"), scalar=W6[t],
                    in1=ohm[:], op0=Alu.mult, op1=Alu.mult,
                    accum_out=ce2c[:, t:t + 1])
                add_dep_helper(i_acc.ins, i_offu.ins, sync=False,
                               reason="CE fills gather gap")
            # V-tree for log-sum-exp (bf16): 28 -> 14 -> 7 -> reduce
            t14 = pool.tile([128, 5, 20, 14], bf16, tag="t14")
            i_t14 = nc.vector.tensor_tensor(out=t14[:], in0=e3r[:, :, :, 0:14],
                                            in1=e3r[:, :, :, 14:28],
                                            op=Alu.add)
            add_dep_helper(i_t14.ins, i_offu.ins, sync=False,
                           reason="CE fills gather gap")
            t7 = pool.tile([128, 5, 20, 7], bf16, tag="t7")
            nc.vector.tensor_tensor(out=t7[:], in0=t14[:, :, :, 0:7],
                                    in1=t14[:, :, :, 7:14], op=Alu.add)
            se6 = pool.tile([128, 6, 20], f32, tag="se6")
            nc.vector.tensor_reduce(out=se6[:, 0:5], in_=t7[:], axis=AX.X,
                                    op=Alu.add)
            nc.vector.tensor_copy(out=se6[:, 5], in_=se5[:])
            lse = pool.tile([128, 6, 20], f32, tag="lse")
            nc.scalar.activation(out=lse[:].rearrange("p a b -> p (a b)"),
                                 in_=se6[:].rearrange("p a b -> p (a b)"),
                                 func=Act.Ln)
            partials = pool.tile([128, 4], f32, tag="partials")

            # ---------------- S3: windowed softmax/huber ----------------
            den = pool.tile([128, NION], fp16, tag="den")
            hubn = pool.tile([128, NION], fp16, tag="hubn")
            iwn = pool.tile([128, NION], fp16, tag="iwn")
            prev_last = None
            for h0, sl in ((0, slice(0, H1)), (1, slice(H1, NION))):
                n = sl.stop - sl.start
                cm = cmpt[:, sl].rearrange("p a (r m) -> p a r m", r=2)
                og = cm[:, :, :, 0:MAXP]
                ig = cm[:, :, :, MAXP:RECW]
                theoB = theo[:, sl][:, :, None, None].broadcast_to(
                    [128, n, 2, MAXP])
                d0 = pool.tile([128, n, 2, MAXP], fp16, tag=f"d0_{h0}")
                i_d0 = nc.vector.tensor_tensor(out=d0[:], in0=og, in1=theoB,
                                               op=Alu.subtract)
                if prev_last is not None:
                    add_dep_helper(i_d0.ins, prev_last.ins, sync=False,
                                   reason="finish S3 half1 before half2")
                dd = pool.tile([128, n, 2, MAXP], fp16, tag=f"dd_{h0}")
                nc.scalar.activation(out=dd[:].rearrange("p a r m -> p (a r m)"),
                                     in_=d0[:].rearrange("p a r m -> p (a r m)"),
                                     func=Act.Abs)
                ee = pool.tile([128, n, 2, MAXP], fp16, tag=f"ee_{h0}")
                nc.scalar.activation(out=ee[:].rearrange("p a r m -> p (a r m)"),
                                     in_=dd[:].rearrange("p a r m -> p (a r m)"),
                                     func=Act.Exp, scale=-1.0 / TEMP)
                mw = pool.tile([128, n, 2, MAXP], fp16, tag=f"mw_{h0}")
                nc.vector.tensor_scalar(out=mw[:], in0=dd[:],
                                        scalar1=MASS_TOL, scalar2=None,
                                        op0=Alu.is_lt)
                ew = pool.tile([128, n, 2, MAXP], fp16, tag=f"ew_{h0}")
                nc.vector.tensor_tensor(out=ew[:], in0=mw[:], in1=ee[:],
                                        op=Alu.mult)
                with nc.allow_low_precision(reason="short fp16 window sums"):
                    nc.vector.tensor_reduce(
                        out=den[:, sl],
                        in_=ew[:].rearrange("p a r m -> p a (r m)"),
                        axis=AX.X, op=Alu.add)
                c1 = pool.tile([128, n, 2, MAXP], fp16, tag=f"c1_{h0}")
                nc.vector.tensor_scalar(out=c1[:], in0=dd[:], scalar1=HUB_D,
                                        scalar2=float(np.sqrt(0.5)),
                                        op0=Alu.min, op1=Alu.mult)
                hm = pool.tile([128, n, 2, MAXP], fp16, tag=f"hm_{h0}")
                nc.scalar.activation(out=hm[:].rearrange("p a r m -> p (a r m)"),
                                     in_=c1[:].rearrange("p a r m -> p (a r m)"),
                                     func=Act.Square)
                rr = pool.tile([128, n, 2, MAXP], fp16, tag=f"rr_{h0}")
                nc.vector.tensor_scalar(out=rr[:], in0=dd[:], scalar1=HUB_D,
                                        scalar2=-HUB_D * HUB_D, op0=Alu.mult,
                                        op1=Alu.add)
                rrm = pool.tile([128, n, 2, MAXP], fp16, tag=f"rrm_{h0}")
                nc.vector.tensor_scalar(out=rrm[:], in0=rr[:], scalar1=0.0,
                                        scalar2=HUB_D * (MASS_TOL - HUB_D),
                                        op0=Alu.max, op1=Alu.min)
                hub = pool.tile([128, n, 2, MAXP], fp16, tag=f"hub_{h0}")
                nc.vector.tensor_tensor(out=hub[:], in0=rrm[:], in1=hm[:],
                                        op=Alu.add)
                he = pool.tile([128, n, 2, MAXP], fp16, tag=f"he_{h0}")
                nc.vector.tensor_tensor(out=he[:], in0=ew[:], in1=hub[:],
                                        op=Alu.mult)
                ie = pool.tile([128, n, 2, MAXP], fp16, tag=f"ie_{h0}")
                nc.vector.tensor_tensor(out=ie[:], in0=ew[:], in1=ig,
                                        op=Alu.mult)
                with nc.allow_low_precision(reason="short fp16 window sums"):
                    nc.vector.tensor_reduce(
                        out=hubn[:, sl],
                        in_=he[:].rearrange("p a r m -> p a (r m)"),
                        axis=AX.X, op=Alu.add)
                    prev_last = nc.vector.tensor_reduce(
                        out=iwn[:, sl],
                        in_=ie[:].rearrange("p a r m -> p a (r m)"),
                        axis=AX.X, op=Alu.add)

            # ---------------- S4: per-ion contributions ----------------
            nm = pool.tile([128, NION], f32, tag="nm")
            nc.vector.tensor_scalar(out=nm[:], in0=den[:], scalar1=0.0,
                                    scalar2=None, op0=Alu.is_gt)
            dsafe = pool.tile([128, NION], f32, tag="dsafe")
            nc.vector.tensor_scalar(out=dsafe[:], in0=den[:], scalar1=1e-12,
                                    scalar2=None, op0=Alu.max)
            rec = pool.tile([128, NION], f32, tag="rec")
            nc.vector.reciprocal(out=rec[:], in_=dsafe[:])
            t1 = pool.tile([128, NION], f32, tag="t1")
            nc.vector.tensor_tensor(out=t1[:], in0=hubn[:], in1=iwn[:],
                                    op=Alu.mult)
            t2 = pool.tile([128, NION], f32, tag="t2")
            nc.vector.tensor_tensor(out=t2[:], in0=t1[:], in1=rec[:],
                                    op=Alu.mult)
            t3 = pool.tile([128, NION], f32, tag="t3")
            nc.vector.tensor_tensor(out=t3[:], in0=t2[:], in1=rec[:],
                                    op=Alu.mult)
            junkC = pool.tile([128, NION], f32, tag="junkC")
            nc.vector.scalar_tensor_tensor(
                out=junkC[:], in0=t3[:], scalar=1.0, in1=nm[:],
                op0=Alu.mult, op1=Alu.mult, accum_out=partials[:, 2:3])
            nc.vector.tensor_reduce(out=partials[:, 3:4], in_=nm[:],
                                    axis=AX.X, op=Alu.add)

            # ---------------- CE tail (after S4, tiny) ----------------
            junkA = pool.tile([128, 6, 20], f32, tag="junkA")
            ce1 = pool.tile([128, 1], f32, tag="ce1")
            nc.vector.scalar_tensor_tensor(
                out=junkA[:].rearrange("p a b -> p (a b)"),
                in0=lse[:].rearrange("p a b -> p (a b)"), scalar=1.0,
                in1=wM[:], op0=Alu.mult, op1=Alu.mult, accum_out=ce1[:])
            ce2 = pool.tile([128, 1], f32, tag="ce2")
            nc.vector.tensor_reduce(out=ce2[:], in_=ce2c[:], axis=AX.X,
                                    op=Alu.add)
            nc.vector.tensor_tensor(out=partials[:, 0:1], in0=ce1[:],
                                    in1=ce2[:], op=Alu.subtract)
            nc.vector.memset(partials[:, 1:2], 0.0)

            # ---------------- output ----------------
            nc.sync.dma_start(out=out_d.ap(), in_=partials[:])

    nc.compile()
    return nc


def _get_nc():
    if "nc" not in _cached:
        _cached["nc"] = _build_program()
    return _cached["nc"]


def _host_prep(all_logits, targets, target_mask, observed_masses,
               observed_intensities, peak_mask, aa_masses):
    """Shard + massage inputs into per-core input maps."""
    all_logits = np.ascontiguousarray(all_logits, dtype=np.float32)
    targets = np.asarray(targets, dtype=np.int32)
    tmask = np.asarray(target_mask).astype(np.float32)
    obs = np.asarray(observed_masses, dtype=np.float32)
    inten = np.asarray(observed_intensities, dtype=np.float32)
    pmask = np.asarray(peak_mask).astype(bool)
    aa = np.asarray(aa_masses, dtype=np.float32)

    # ---- uniform slot table: per row, slot k covers [100+k*D, 100+(k+1)*D)
    rows, cols = np.nonzero(pmask)
    m = obs[rows, cols]
    k = np.floor((m - 100.0) / DELTA).astype(np.int64)
    np.clip(k, 0, NSLOT - 1, out=k)
    flat = rows * NSLOT + k
    # masses sorted per row => flat is sorted; rank within each slot:
    first = np.searchsorted(flat, flat, side="left")
    pos = np.arange(flat.size) - first
    if pos.max() >= MAXP:
        raise RuntimeError(f"slot overflow: {pos.max() + 1} peaks in one "
                           f"{DELTA}-Da slot (MAXP={MAXP})")
    slots1 = np.full((B * NSLOT, RECW), 0.0, dtype=np.float32)
    slots1[:, 0:MAXP] = PADM
    slots1[flat, pos] = m
    slots1[flat, MAXP + pos] = inten[rows, cols]
    # row k holds slots [k, k+1] so each gather reads one contiguous row
    slots = np.empty((B * NSLOT, 2 * RECW), dtype=np.float32)
    slots[:, 0:RECW] = slots1
    slots[:-1, RECW:] = slots1[1:]
    slots[-1, RECW:RECW + MAXP] = PADM
    slots[-1, RECW + MAXP:] = 0.0

    # ---- aux: onehot(targets)*mask (bf16) + w_t*mask (f32) + aa (f32)
    oh = (np.arange(V)[None, None, :] == targets[:, :, None])
    nz = (targets != 0)
    mf = tmask
    ohm = (oh * (mf * nz)[:, :, None]).astype(np.float32)  # [B, S, V]
    w6 = (np.arange(1, T + 1, dtype=np.float32) / 21.0)
    wMf = w6[None, :, None] * (mf * nz)[:, None, :]        # [B, T, S]

    def to_bf16_bytes(x):
        u = x.astype(np.float32).view(np.uint32)
        r = ((u >> 16) + ((u >> 15) & 1)).astype(np.uint16)  # round-nearest-ish
        return r

    denom = max(float(mf.sum()), 1.0)

    in_maps = []
    for c in range(NCORES):
        sl = slice(c * BS, (c + 1) * BS)
        lg = all_logits[:, sl]                     # [T, 64, 40, 28]
        # partition p = 64*h + b  (h = s-half)
        lgp = lg.reshape(T, BS, 2, 20, V).transpose(0, 2, 1, 3, 4)
        lgp = np.ascontiguousarray(lgp.reshape(T, 128, 20 * V))
        ohm_c = ohm[sl].reshape(BS, 2, 20, V).transpose(1, 0, 2, 3)
        ohm16 = to_bf16_bytes(ohm_c.reshape(128, 560))
        wM_c = wMf[sl].reshape(BS, T, 2, 20).transpose(2, 0, 1, 3)
        wM_c = np.ascontiguousarray(wM_c.reshape(128, 120), dtype=np.float32)
        aux = np.zeros((128, 1728), dtype=np.uint8)
        aux[:, 0:1120] = ohm16.view(np.uint8).reshape(128, 1120)
        aux[:, 1120:1600] = wM_c.view(np.uint8)
        aux[:, 1600:1712] = np.broadcast_to(
            aa[None, :].view(np.uint8).reshape(1, 112), (128, 112))
        in_maps.append({
            "lg5": np.ascontiguousarray(lgp[5]),
            "lgr": np.ascontiguousarray(lgp[0:5]),
            "aux": aux,
            "slots": np.ascontiguousarray(
                slots.reshape(B, NSLOT, 2 * RECW)[sl].reshape(
                    BS * NSLOT, 2 * RECW)),
        })
    return in_maps, denom


def _combine(results, denom):
    ce_num = 0.0
    sp_num = 0.0
    sp_cnt = 0.0
    for r in results:
        p = r["partials"].astype(np.float64)
        ce_num += p[:, 0].sum()
        sp_num += p[:, 2].sum()
        sp_cnt += p[:, 3].sum()
    ce = ce_num / denom
    spec = sp_num / max(sp_cnt, 1.0)
    return np.float32(CE_W * ce + SPEC_W * spec)


def kernel(**inputs) -> np.ndarray:
    from concourse.bass_utils import run_bass_kernel_spmd

    nc = _get_nc()
    in_maps, denom = _host_prep(**inputs)
    res = run_bass_kernel_spmd(nc, in_maps, core_ids=list(range(NCORES)))
    return _combine(res.results, denom)


# revision 12
# speedup vs baseline: 1.1800x; 1.0203x over previous
"""Trainium2 Bass kernel for nn_CombinedLoss (deep-supervision CE + spectrum loss).

V2 strategy (data parallel over batch B=512 -> 64 rows per core; partition
p = 64*h + b, h = s-half):

Spectrum (critical path): logits[5] DMA'd first -> exp/se/expected residue
mass [128,20] -> in-partition cumsum (tensor_tensor_scan) -> cross-partition
fixup via two tiny PE matmuls -> theo ion masses [128,57] -> arithmetic slot
index (host pre-bins peaks into uniform 1.25-Da slots of <=5 peaks) -> one
indirect-DMA gather of 2 consecutive slots per ion (no on-chip searchsorted)
-> windowed softmax/huber on [128, 57*10] in fp16 where safe.

CE (fills the gather gap): host packs one-hot(targets)*mask (bf16) and
w_t*mask (f32) into an aux tensor; on-chip work is exp (ACT, bf16), a
log-sum-exp V-tree in bf16, and 6 multiply-accumulate ops on raw logits.

One activation-table preload (natural_log_exp_and_others) covers
Exp/Ln/Abs/Square, so no mid-kernel table switches.
"""

import os
import sys

import numpy as np

for _p in ("/opt/trn_rl_repo", "/root/.axon_site/_ro/trn_rl_repo"):
    if os.path.isdir(_p) and _p not in sys.path:
        sys.path.append(_p)

T, B, S, V = 6, 512, 40, 28
N_PEAKS = 512
NCORES = 8
BS = B // NCORES          # 64 batch rows per core
PROTON = 1.007276
WATER = 18.010565
CO = 27.994915
MASS_TOL = 0.5
TEMP = 0.1
HUB_D = 0.2
CE_W = 1.0
SPEC_W = 0.1

NION = 57                 # ion slots per partition (19 per family)
DELTA = 1.25              # slot width in Da
MAXP = 5                  # max peaks per slot (asserted host-side)
NSLOT = 1528              # slots per row (covers 100..2010 + pad)
KMAX = float(NSLOT - 2)
RECW = 2 * MAXP           # f32 elems per slot record (5 mass + 5 inten)
WG = 2 * MAXP             # gathered candidate peaks per ion (2 slots)
PADM = 30000.0            # pad mass (stays finite in fp16)
H1 = 28                   # ions in first gather half

_cached = {}


def _build_program():
    import concourse.bass as bass
    import concourse.bacc as bacc
    import concourse.mybir as mybir
    import concourse.tile as tile

    dt = mybir.dt
    Alu = mybir.AluOpType
    Act = mybir.ActivationFunctionType
    AX = mybir.AxisListType

    nc = bacc.Bacc("TRN2", target_bir_lowering=False, debug=False,
                   num_devices=NCORES)

    f32, bf16, fp16 = dt.float32, dt.bfloat16, dt.float16

    lg5_d = nc.dram_tensor("lg5", [128, 20 * V], f32, kind="ExternalInput")
    lgr_d = nc.dram_tensor("lgr", [5, 128, 20 * V], f32, kind="ExternalInput")
    aux_d = nc.dram_tensor("aux", [128, 1728], dt.uint8, kind="ExternalInput")
    slots_d = nc.dram_tensor("slots", [BS * NSLOT, 2 * RECW], f32,
                             kind="ExternalInput")
    out_d = nc.dram_tensor("partials", [128, 4], f32, kind="ExternalOutput")

    with tile.TileContext(nc) as tc:
        with tc.tile_pool(name="main", bufs=1) as pool, \
             tc.tile_pool(name="ps", bufs=1, space="PSUM") as psp:

            # ---- act table preload: set 6 = natural_log_exp_and_others ----
            ld = mybir.InstLoadActFuncSet(
                name=nc.get_next_instruction_name(), ins=[], outs=[],
                act_func_set_id=6)
            nc.scalar.add_instruction(ld)

            # ---------------- input DMAs (SP seq; t5 first) ----------------
            lg5 = pool.tile([128, 20, V], f32, tag="lg5")
            nc.sync.dma_start(out=lg5[:].rearrange("p a b -> p (a b)"),
                              in_=lg5_d.ap())
            aux = pool.tile([128, 1728], dt.uint8, tag="aux")
            nc.sync.dma_start(out=aux[:], in_=aux_d.ap())
            lgr = pool.tile([128, 5, 20, V], f32, tag="lgr")
            nc.sync.dma_start(out=lgr[:].rearrange("p t a b -> p t (a b)"),
                              in_=lgr_d.ap().rearrange("t p n -> p t n"))

            ohm = aux[:, 0:1120].bitcast(bf16)            # [128, 560]
            wM = aux[:, 1120:1600].bitcast(f32)           # [128, 120]
            aar = aux[:, 1600:1712].bitcast(f32)          # [128, 28]

            # ---------------- early constants (Pool/DVE) ----------------
            iota_p = pool.tile([128, 1], dt.int32, tag="iota_p")
            nc.gpsimd.iota(iota_p[:], pattern=[[0, 1]], channel_multiplier=1)
            pp_f = pool.tile([128, 1], f32, tag="pp_f")
            nc.vector.tensor_copy(out=pp_f[:], in_=iota_p[:])
            # b = p mod 64 ; base = b * NSLOT (slot-unit gather base)
            ge64 = pool.tile([128, 1], f32, tag="ge64")
            nc.vector.tensor_scalar(out=ge64[:], in0=pp_f[:], scalar1=63.5,
                                    scalar2=64.0, op0=Alu.is_gt, op1=Alu.mult)
            b_f = pool.tile([128, 1], f32, tag="b_f")
            nc.vector.tensor_tensor(out=b_f[:], in0=pp_f[:], in1=ge64[:],
                                    op=Alu.subtract)
            base_f = pool.tile([128, 1], f32, tag="base_f")
            nc.vector.tensor_scalar(out=base_f[:], in0=b_f[:],
                                    scalar1=float(NSLOT), scalar2=None,
                                    op0=Alu.mult)
            # selector matrices for cross-partition cumsum fixup
            iota_n = pool.tile([128, 128], dt.int32, tag="iota_n")
            nc.gpsimd.iota(iota_n[:], pattern=[[1, 128]], channel_multiplier=0)
            iota_nf = pool.tile([128, 128], f32, tag="iota_nf")
            nc.vector.tensor_copy(out=iota_nf[:], in_=iota_n[:])
            p64 = pool.tile([128, 1], f32, tag="p64")
            nc.vector.tensor_scalar(out=p64[:], in0=pp_f[:], scalar1=64.0,
                                    scalar2=None, op0=Alu.add)
            sel1 = pool.tile([128, 128], f32, tag="sel1")
            nc.vector.tensor_scalar(out=sel1[:], in0=iota_nf[:],
                                    scalar1=p64[:], scalar2=None,
                                    op0=Alu.is_equal)
            # sel2[p', n] = 1 iff n == p' - 64  (h1 row p' feeds h0 column n)
            pm64 = pool.tile([128, 1], f32, tag="pm64")
            nc.vector.tensor_scalar(out=pm64[:], in0=pp_f[:], scalar1=-64.0,
                                    scalar2=None, op0=Alu.add)
            sel2 = pool.tile([128, 128], f32, tag="sel2")
            nc.vector.tensor_scalar(out=sel2[:], in0=iota_nf[:],
                                    scalar1=pm64[:], scalar2=None,
                                    op0=Alu.is_equal)

            theo = pool.tile([128, NION], f32, tag="theo")
            nc.vector.memset(theo[:], -PADM)

            # ---------------- spectrum front (t=5) ----------------
            e3_5 = pool.tile([128, 20, V], f32, tag="e3_5")
            nc.scalar.activation(out=e3_5[:], in_=lg5[:], func=Act.Exp)
            se5 = pool.tile([128, 20], f32, tag="se5")
            nc.vector.tensor_reduce(out=se5[:], in_=e3_5[:], axis=AX.X,
                                    op=Alu.add)
            pe5 = pool.tile([128, 20], f32, tag="pe5")
            nc.vector.reciprocal(out=pe5[:], in_=se5[:])
            prod5 = pool.tile([128, 20, V], f32, tag="prod5")
            nc.gpsimd.tensor_tensor(
                out=prod5[:], in0=e3_5[:],
                in1=aar[:, None, :].broadcast_to([128, 20, V]), op=Alu.mult)
            nume = pool.tile([128, 20], f32, tag="nume")
            nc.vector.tensor_reduce(out=nume[:], in_=prod5[:], axis=AX.X,
                                    op=Alu.add)
            expected = pool.tile([128, 20], f32, tag="expected")
            nc.vector.tensor_tensor(out=expected[:], in0=nume[:], in1=pe5[:],
                                    op=Alu.mult)
            # residues are s=1..38: zero s=0 (h0 col 0) and s=39 (h1 col 19)
            nc.vector.memset(expected[0:64, 0:1], 0.0)
            nc.vector.memset(expected[64:128, 19:20], 0.0)

            # in-partition cumsum over the 20 s-slots
            zer20 = pool.tile([128, 20], f32, tag="zer20")
            nc.vector.memset(zer20[:], 0.0)
            cum = pool.tile([128, 20], f32, tag="cum")
            nc.vector.tensor_tensor_scan(out=cum[:], data0=expected[:],
                                         data1=zer20[:], initial=0.0,
                                         op0=Alu.add, op1=Alu.add)

            # cross-partition fixups: fix[:,0] = prev-half total (h1 rows),
            # fix[:,1] = per-b grand total
            fix_ps = psp.tile([128, 2], f32, tag="fix_ps")
            nc.tensor.matmul(out=fix_ps[:, 0:1], lhsT=sel1[:],
                             rhs=cum[:, 19:20], start=True, stop=True)
            nc.tensor.matmul(out=fix_ps[:, 1:2], lhsT=sel2[:],
                             rhs=cum[:, 19:20], start=True, stop=True)
            fix = pool.tile([128, 2], f32, tag="fix")
            nc.scalar.copy(out=fix[:], in_=fix_ps[:])
            cfull = pool.tile([128, 20], f32, tag="cfull")
            nc.vector.tensor_scalar(out=cfull[:], in0=cum[:],
                                    scalar1=fix[:, 0:1], scalar2=None,
                                    op0=Alu.add)
            # grand total c37 = own local sum + partner-half local sum
            tsum = pool.tile([128, 1], f32, tag="tsum")
            nc.vector.tensor_tensor(out=tsum[:], in0=fix[:, 0:1],
                                    in1=fix[:, 1:2], op=Alu.add)
            totWP = pool.tile([128, 1], f32, tag="totWP")
            nc.vector.tensor_scalar(out=totWP[:], in0=tsum[:],
                                    scalar1=cum[:, 19:20],
                                    scalar2=WATER + PROTON,
                                    op0=Alu.add, op1=Alu.add)

            # theo[0:19]=b-ions, [19:38]=y-ions, [38:57]=a-ions
            nc.vector.tensor_scalar(out=theo[0:64, 0:19],
                                    in0=cfull[0:64, 1:20], scalar1=PROTON,
                                    scalar2=None, op0=Alu.add)
            nc.vector.tensor_scalar(out=theo[64:128, 0:18],
                                    in0=cfull[64:128, 0:18], scalar1=PROTON,
                                    scalar2=None, op0=Alu.add)
            # y_q = totWP - c_{q-1}: h0 uses cfull[:, 0:19] (col 0 == 0);
            # h1 slot 0 (q=19) uses c_18 = fix1, slots 1..17 use cfull[0:17]
            nc.vector.tensor_scalar(out=theo[0:64, 19:38],
                                    in0=cfull[0:64, 0:19], scalar1=-1.0,
                                    scalar2=totWP[0:64], op0=Alu.mult,
                                    op1=Alu.add)
            nc.vector.tensor_scalar(out=theo[64:128, 19:20],
                                    in0=fix[64:128, 0:1], scalar1=-1.0,
                                    scalar2=totWP[64:128], op0=Alu.mult,
                                    op1=Alu.add)
            nc.vector.tensor_scalar(out=theo[64:128, 20:37],
                                    in0=cfull[64:128, 0:17], scalar1=-1.0,
                                    scalar2=totWP[64:128], op0=Alu.mult,
                                    op1=Alu.add)
            nc.vector.tensor_scalar(out=theo[0:64, 38:57],
                                    in0=cfull[0:64, 1:20],
                                    scalar1=PROTON - CO, scalar2=None,
                                    op0=Alu.add)
            nc.vector.tensor_scalar(out=theo[64:128, 38:56],
                                    in0=cfull[64:128, 0:18],
                                    scalar1=PROTON - CO, scalar2=None,
                                    op0=Alu.add)

            # ---------------- slot index + gather ----------------
            kf = pool.tile([128, NION], f32, tag="kf")
            nc.vector.tensor_scalar(out=kf[:], in0=theo[:],
                                    scalar1=1.0 / DELTA,
                                    scalar2=-(100.0 + MASS_TOL) / DELTA,
                                    op0=Alu.mult, op1=Alu.add)
            kc = pool.tile([128, NION], f32, tag="kc")
            nc.vector.tensor_scalar(out=kc[:], in0=kf[:], scalar1=0.0,
                                    scalar2=KMAX, op0=Alu.max, op1=Alu.min)
            off_f = pool.tile([128, NION], f32, tag="off_f")
            nc.vector.tensor_scalar(out=off_f[:], in0=kc[:],
                                    scalar1=base_f[:], scalar2=None,
                                    op0=Alu.add)
            off_u = pool.tile([128, NION], dt.uint32, tag="off_u")
            i_offu = nc.vector.tensor_copy(out=off_u[:], in_=off_f[:])

            cmpt = pool.tile([128, NION, 2 * RECW], f32, tag="cmpt")
            gathers = []
            for h0, sl in ((0, slice(0, H1)), (1, slice(H1, NION))):
                g = nc.gpsimd.indirect_dma_start(
                    out=cmpt[:, sl].rearrange("p a b -> p (a b)"),
                    out_offset=None,
                    in_=slots_d.ap(),
                    in_offset=bass.IndirectOffsetOnAxis(ap=off_u[:, sl],
                                                        axis=0))
                gathers.append(g)

            # ---------------- CE (fills the gather gap) ----------------
            from concourse.tile import add_dep_helper
            e3r = pool.tile([128, 5, 20, V], bf16, tag="e3r")
            nc.scalar.activation(
                out=e3r[:].rearrange("p t a b -> p (t a b)"),
                in_=lgr[:].rearrange("p t a b -> p (t a b)"), func=Act.Exp)
            # x-part: sum_t w_t * sum_{s,v} logits * onehot*mask
            W6 = [(i + 1) / 21.0 for i in range(T)]
            ce2c = pool.tile([128, 6], f32, tag="ce2c")
            junkB = pool.tile([128, 20, V], f32, tag="junkB")
            for t in range(T):
                srcl = lg5[:] if t == 5 else lgr[:, t]
                i_acc = nc.vector.scalar_tensor_tensor(
                    out=junkB[:].rearrange("p a b -> p (a b)"),
                    in0=srcl.rearrange("p a b -> p (a b)"), scalar=W6[t],
                    in1=ohm[:], op0=Alu.mult, op1=Alu.mult,
                    accum_out=ce2c[:, t:t + 1])
                add_dep_helper(i_acc.ins, i_offu.ins, sync=False,
                               reason="CE fills gather gap")
            # V-tree for log-sum-exp (bf16): 28 -> 14 -> 7 -> reduce
            t14 = pool.tile([128, 5, 20, 14], bf16, tag="t14")
            i_t14 = nc.vector.tensor_tensor(out=t14[:], in0=e3r[:, :, :, 0:14],
                                            in1=e3r[:, :, :, 14:28],
                                            op=Alu.add)
            add_dep_helper(i_t14.ins, i_offu.ins, sync=False,
                           reason="CE fills gather gap")
            t7 = pool.tile([128, 5, 20, 7], bf16, tag="t7")
            nc.vector.tensor_tensor(out=t7[:], in0=t14[:, :, :, 0:7],
                                    in1=t14[:, :, :, 7:14], op=Alu.add)
            se6 = pool.tile([128, 6, 20], f32, tag="se6")
            nc.vector.tensor_reduce(out=se6[:, 0:5], in_=t7[:], axis=AX.X,
                                    op=Alu.add)
            nc.vector.tensor_copy(out=se6[:, 5], in_=se5[:])
            lse = pool.tile([128, 6, 20], f32, tag="lse")
            nc.scalar.activation(out=lse[:].rearrange("p a b -> p (a b)"),
                                 in_=se6[:].rearrange("p a b -> p (a b)"),
                                 func=Act.Ln)
            partials = pool.tile([128, 4], f32, tag="partials")

            # ---------------- S3: windowed softmax/huber ----------------
            den = pool.tile([128, NION], fp16, tag="den")
            hubn = pool.tile([128, NION], fp16, tag="hubn")
            iwn = pool.tile([128, NION], fp16, tag="iwn")
            prev_last = None
            for h0, sl in ((0, slice(0, H1)), (1, slice(H1, NION))):
                n = sl.stop - sl.start
                cm = cmpt[:, sl].rearrange("p a (r m) -> p a r m", r=2)
                og = cm[:, :, :, 0:MAXP]
                ig = cm[:, :, :, MAXP:RECW]
                theoB = theo[:, sl][:, :, None, None].broadcast_to(
                    [128, n, 2, MAXP])
                d0 = pool.tile([128, n, 2, MAXP], fp16, tag=f"d0_{h0}")
                i_d0 = nc.vector.tensor_tensor(out=d0[:], in0=og, in1=theoB,
                                               op=Alu.subtract)
                if prev_last is not None:
                    add_dep_helper(i_d0.ins, prev_last.ins, sync=False,
                                   reason="finish S3 half1 before half2")
                dd = pool.tile([128, n, 2, MAXP], fp16, tag=f"dd_{h0}")
                nc.vector.scalar_tensor_tensor(out=dd[:], in0=d0[:],
                                               scalar=-1.0, in1=d0[:],
                                               op0=Alu.mult, op1=Alu.max)
                ee = pool.tile([128, n, 2, MAXP], fp16, tag=f"ee_{h0}")
                nc.scalar.activation(out=ee[:].rearrange("p a r m -> p (a r m)"),
                                     in_=dd[:].rearrange("p a r m -> p (a r m)"),
                                     func=Act.Exp, scale=-1.0 / TEMP)
                mw = pool.tile([128, n, 2, MAXP], fp16, tag=f"mw_{h0}")
                nc.vector.tensor_scalar(out=mw[:], in0=dd[:],
                                        scalar1=MASS_TOL, scalar2=None,
                                        op0=Alu.is_lt)
                ew = pool.tile([128, n, 2, MAXP], fp16, tag=f"ew_{h0}")
                nc.vector.tensor_tensor(out=ew[:], in0=mw[:], in1=ee[:],
                                        op=Alu.mult)
                with nc.allow_low_precision(reason="short fp16 window sums"):
                    nc.vector.tensor_reduce(
                        out=den[:, sl],
                        in_=ew[:].rearrange("p a r m -> p a (r m)"),
                        axis=AX.X, op=Alu.add)
                c1 = pool.tile([128, n, 2, MAXP], fp16, tag=f"c1_{h0}")
                nc.vector.tensor_scalar(out=c1[:], in0=dd[:], scalar1=HUB_D,
                                        scalar2=float(np.sqrt(0.5)),
                                        op0=Alu.min, op1=Alu.mult)
                hm = pool.tile([128, n, 2, MAXP], fp16, tag=f"hm_{h0}")
                nc.vector.tensor_tensor(out=hm[:], in0=c1[:], in1=c1[:],
                                        op=Alu.mult)
                rr = pool.tile([128, n, 2, MAXP], fp16, tag=f"rr_{h0}")
                nc.vector.tensor_scalar(out=rr[:], in0=dd[:], scalar1=HUB_D,
                                        scalar2=-HUB_D * HUB_D, op0=Alu.mult,
                                        op1=Alu.add)
                rrm = pool.tile([128, n, 2, MAXP], fp16, tag=f"rrm_{h0}")
                nc.vector.tensor_scalar(out=rrm[:], in0=rr[:], scalar1=0.0,
                                        scalar2=HUB_D * (MASS_TOL - HUB_D),
                                        op0=Alu.max, op1=Alu.min)
                hub = pool.tile([128, n, 2, MAXP], fp16, tag=f"hub_{h0}")
                nc.vector.tensor_tensor(out=hub[:], in0=rrm[:], in1=hm[:],
                                        op=Alu.add)
                he = pool.tile([128, n, 2, MAXP], fp16, tag=f"he_{h0}")
                nc.vector.tensor_tensor(out=he[:], in0=ew[:], in1=hub[:],
                                        op=Alu.mult)
                ie = pool.tile([128, n, 2, MAXP], fp16, tag=f"ie_{h0}")
                nc.vector.tensor_tensor(out=ie[:], in0=ew[:], in1=ig,
                                        op=Alu.mult)
                with nc.allow_low_precision(reason="short fp16 window sums"):
                    nc.vector.tensor_reduce(
                        out=hubn[:, sl],
                        in_=he[:].rearrange("p a r m -> p a (r m)"),
                        axis=AX.X, op=Alu.add)
                    prev_last = nc.vector.tensor_reduce(
                        out=iwn[:, sl],
                        in_=ie[:].rearrange("p a r m -> p a (r m)"),
                        axis=AX.X, op=Alu.add)

            # ---------------- S4: per-ion contributions ----------------
            nm = pool.tile([128, NION], f32, tag="nm")
            nc.vector.tensor_scalar(out=nm[:], in0=den[:], scalar1=0.0,
                                    scalar2=None, op0=Alu.is_gt)
            dsafe = pool.tile([128, NION], f32, tag="dsafe")
            nc.vector.tensor_scalar(out=dsafe[:], in0=den[:], scalar1=1e-12,
                                    scalar2=None, op0=Alu.max)
            rec = pool.tile([128, NION], f32, tag="rec")
            nc.vector.reciprocal(out=rec[:], in_=dsafe[:])
            t1 = pool.tile([128, NION], f32, tag="t1")
            nc.vector.tensor_tensor(out=t1[:], in0=hubn[:], in1=iwn[:],
                                    op=Alu.mult)
            t2 = pool.tile([128, NION], f32, tag="t2")
            nc.vector.tensor_tensor(out=t2[:], in0=t1[:], in1=rec[:],
                                    op=Alu.mult)
            t3 = pool.tile([128, NION], f32, tag="t3")
            nc.vector.tensor_tensor(out=t3[:], in0=t2[:], in1=rec[:],
                                    op=Alu.mult)
            junkC = pool.tile([128, NION], f32, tag="junkC")
            nc.vector.scalar_tensor_tensor(
                out=junkC[:], in0=t3[:], scalar=1.0, in1=nm[:],
                op0=Alu.mult, op1=Alu.mult, accum_out=partials[:, 2:3])
            nc.vector.tensor_reduce(out=partials[:, 3:4], in_=nm[:],
                                    axis=AX.X, op=Alu.add)

            # ---------------- CE tail (after S4, tiny) ----------------
            junkA = pool.tile([128, 6, 20], f32, tag="junkA")
            ce1 = pool.tile([128, 1], f32, tag="ce1")
            nc.vector.scalar_tensor_tensor(
                out=junkA[:].rearrange("p a b -> p (a b)"),
                in0=lse[:].rearrange("p a b -> p (a b)"), scalar=1.0,
                in1=wM[:], op0=Alu.mult, op1=Alu.mult, accum_out=ce1[:])
            ce2 = pool.tile([128, 1], f32, tag="ce2")
            nc.vector.tensor_reduce(out=ce2[:], in_=ce2c[:], axis=AX.X,
                                    op=Alu.add)
            nc.vector.tensor_tensor(out=partials[:, 0:1], in0=ce1[:],
                                    in1=ce2[:], op=Alu.subtract)
            nc.vector.memset(partials[:, 1:2], 0.0)

            # ---------------- output ----------------
            nc.sync.dma_start(out=out_d.ap(), in_=partials[:])

    nc.compile()
    return nc


def _get_nc():
    if "nc" not in _cached:
        _cached["nc"] = _build_program()
    return _cached["nc"]


def _host_prep(all_logits, targets, target_mask, observed_masses,
               observed_intensities, peak_mask, aa_masses):
    """Shard + massage inputs into per-core input maps."""
    all_logits = np.ascontiguousarray(all_logits, dtype=np.float32)
    targets = np.asarray(targets, dtype=np.int32)
    tmask = np.asarray(target_mask).astype(np.float32)
    obs = np.asarray(observed_masses, dtype=np.float32)
    inten = np.asarray(observed_intensities, dtype=np.float32)
    pmask = np.asarray(peak_mask).astype(bool)
    aa = np.asarray(aa_masses, dtype=np.float32)

    # ---- uniform slot table: per row, slot k covers [100+k*D, 100+(k+1)*D)
    rows, cols = np.nonzero(pmask)
    m = obs[rows, cols]
    k = np.floor((m - 100.0) / DELTA).astype(np.int64)
    np.clip(k, 0, NSLOT - 1, out=k)
    flat = rows * NSLOT + k
    # masses sorted per row => flat is sorted; rank within each slot:
    first = np.searchsorted(flat, flat, side="left")
    pos = np.arange(flat.size) - first
    if pos.max() >= MAXP:
        raise RuntimeError(f"slot overflow: {pos.max() + 1} peaks in one "
                           f"{DELTA}-Da slot (MAXP={MAXP})")
    slots1 = np.full((B * NSLOT, RECW), 0.0, dtype=np.float32)
    slots1[:, 0:MAXP] = PADM
    slots1[flat, pos] = m
    slots1[flat, MAXP + pos] = inten[rows, cols]
    # row k holds slots [k, k+1] so each gather reads one contiguous row
    slots = np.empty((B * NSLOT, 2 * RECW), dtype=np.float32)
    slots[:, 0:RECW] = slots1
    slots[:-1, RECW:] = slots1[1:]
    slots[-1, RECW:RECW + MAXP] = PADM
    slots[-1, RECW + MAXP:] = 0.0

    # ---- aux: onehot(targets)*mask (bf16) + w_t*mask (f32) + aa (f32)
    oh = (np.arange(V)[None, None, :] == targets[:, :, None])
    nz = (targets != 0)
    mf = tmask
    ohm = (oh * (mf * nz)[:, :, None]).astype(np.float32)  # [B, S, V]
    w6 = (np.arange(1, T + 1, dtype=np.float32) / 21.0)
    wMf = w6[None, :, None] * (mf * nz)[:, None, :]        # [B, T, S]

    def to_bf16_bytes(x):
        u = x.astype(np.float32).view(np.uint32)
        r = ((u >> 16) + ((u >> 15) & 1)).astype(np.uint16)  # round-nearest-ish
        return r

    denom = max(float(mf.sum()), 1.0)

    in_maps = []
    for c in range(NCORES):
        sl = slice(c * BS, (c + 1) * BS)
        lg = all_logits[:, sl]                     # [T, 64, 40, 28]
        # partition p = 64*h + b  (h = s-half)
        lgp = lg.reshape(T, BS, 2, 20, V).transpose(0, 2, 1, 3, 4)
        lgp = np.ascontiguousarray(lgp.reshape(T, 128, 20 * V))
        ohm_c = ohm[sl].reshape(BS, 2, 20, V).transpose(1, 0, 2, 3)
        ohm16 = to_bf16_bytes(ohm_c.reshape(128, 560))
        wM_c = wMf[sl].reshape(BS, T, 2, 20).transpose(2, 0, 1, 3)
        wM_c = np.ascontiguousarray(wM_c.reshape(128, 120), dtype=np.float32)
        aux = np.zeros((128, 1728), dtype=np.uint8)
        aux[:, 0:1120] = ohm16.view(np.uint8).reshape(128, 1120)
        aux[:, 1120:1600] = wM_c.view(np.uint8)
        aux[:, 1600:1712] = np.broadcast_to(
            aa[None, :].view(np.uint8).reshape(1, 112), (128, 112))
        in_maps.append({
            "lg5": np.ascontiguousarray(lgp[5]),
            "lgr": np.ascontiguousarray(lgp[0:5]),
            "aux": aux,
            "slots": np.ascontiguousarray(
                slots.reshape(B, NSLOT, 2 * RECW)[sl].reshape(
                    BS * NSLOT, 2 * RECW)),
        })
    return in_maps, denom


def _combine(results, denom):
    ce_num = 0.0
    sp_num = 0.0
    sp_cnt = 0.0
    for r in results:
        p = r["partials"].astype(np.float64)
        ce_num += p[:, 0].sum()
        sp_num += p[:, 2].sum()
        sp_cnt += p[:, 3].sum()
    ce = ce_num / denom
    spec = sp_num / max(sp_cnt, 1.0)
    return np.float32(CE_W * ce + SPEC_W * spec)


def kernel(**inputs) -> np.ndarray:
    from concourse.bass_utils import run_bass_kernel_spmd

    nc = _get_nc()
    in_maps, denom = _host_prep(**inputs)
    res = run_bass_kernel_spmd(nc, in_maps, core_ids=list(range(NCORES)))
    return _combine(res.results, denom)
